# revision 29
# baseline (speedup 1.0000x reference)
"""Trainium2 Bass kernel for nn_CreepDiffusionTrainer.

The reference builds dense NxN (6144x6144) pairwise coupling / laplacian
matrices from points that lie on a uniform 96x64 grid.  On that grid every
pairwise quantity depends only on the integer offset (di, dj), so the dense
matvecs are exactly 2D convolutions with small stencils:

  - coupling (dist < 50):  68 taps inside a 9x9 window
  - laplacian (dist < 30):  24 taps inside a 5x5 window

(nearest excluded offsets are >= 0.28 px away from the thresholds, far beyond
f32 rounding of the coordinates, so the masks are identical).

The kernel keeps the state in a transposed "V-frame" layout: SBUF tiles of
[64 partitions = x-column j, 96 free = y-row i], with the x- and y-systems
stacked into one [128, 96] tile.  Each conv is 9 PSUM-accumulated matmuls
out[j',i] += sum_j B_di[j, j'] * state[j, i+di] with constant banded weight
matrices B_di (the dj-part of the stencil) and free-dim-shifted reads of a
zero-padded state tile (the di-part).  Row sums, the creep/elongation init
and all scalar schedules are folded into small constant tensors on the host.

The same program runs replicated on all 8 NeuronCores (the whole problem is
~30us of latency-bound work; sharding 96 rows across cores would only add
per-iteration collective latency), and core 0's output is returned.

If the points ever do NOT form the expected uniform grid, a dense numpy
fallback that mirrors the reference math exactly is used instead.
"""

import numpy as np

IMG_H, IMG_W = 960, 640
NY, NX = 96, 64
N = NY * NX
DT = 1.0
ELONGATION = 0.15
COUPLING = 0.3
DIFFUSION = 0.1
EM = 1.0
VISC = 10.0
HX = (IMG_W - 1) / (NX - 1)
HY = (IMG_H - 1) / (NY - 1)
R_F = 4   # force stencil radius (dist < 50)
R_L = 2   # laplacian stencil radius (dist < 30)
N_CORES = 8


# ---------------------------------------------------------------------------
# host-side constant construction (all f64, cast to f32 at the end)
# ---------------------------------------------------------------------------

def _grid_points():
    xs = np.linspace(0.0, IMG_W - 1, NX)
    ys = np.linspace(0.0, IMG_H - 1, NY)
    Y, X = np.meshgrid(ys, xs, indexing="ij")
    return np.stack([X.ravel(), Y.ravel()], axis=-1)


def _correlate2d_zero(U, K, R):
    out = np.zeros_like(U)
    ny, nx = U.shape
    for di in range(-R, R + 1):
        for dj in range(-R, R + 1):
            k = K[di + R, dj + R]
            if k == 0.0:
                continue
            i0, i1 = max(0, -di), ny - max(0, di)
            j0, j1 = max(0, -dj), nx - max(0, dj)
            out[i0:i1, j0:j1] += k * U[i0 + di:i1 + di, j0 + dj:j1 + dj]
    return out


def _stencil_tables(tf):
    pc = COUPLING * (1.0 + tf * 0.5)
    wc = pc * EM * (1.0 + tf)
    Kx = np.zeros((2 * R_F + 1, 2 * R_F + 1))
    Ky = np.zeros_like(Kx)
    KL = np.zeros((2 * R_L + 1, 2 * R_L + 1))
    for di in range(-R_F, R_F + 1):
        for dj in range(-R_F, R_F + 1):
            d = float(np.hypot(di * HY, dj * HX))
            if 0.0 < d < 50.0:
                w = wc * np.exp(-d / 20.0)
                Kx[di + R_F, dj + R_F] = w * (dj * HX) / (d * d)
                Ky[di + R_F, dj + R_F] = w * (di * HY) / (d * d)
            if 0.0 < d < 30.0 and abs(di) <= R_L and abs(dj) <= R_L:
                KL[di + R_L, dj + R_L] = 1.0 / (d + 1e-6)
    ones = np.ones((NY, NX))
    rowAx = _correlate2d_zero(ones, Kx, R_F)
    rowAy = _correlate2d_zero(ones, Ky, R_F)
    rowL = _correlate2d_zero(ones, KL, R_L)
    return Kx, Ky, KL, rowAx, rowAy, rowL


def _banded(K1d_getter, n, band_r):
    """B[j, j'] = K(j - j') as an n x n matrix."""
    B = np.zeros((n, n))
    for j in range(n):
        for jp in range(max(0, j - band_r), min(n, j + band_r + 1)):
            B[j, jp] = K1d_getter(j - jp)
    return B


def _host_constants(points, boundary_flags, moisture_u, t, total_steps):
    tf = float(t) / float(total_steps)
    py = points[:, 1].astype(np.float64)
    center_y = py.mean()

    # creep / elongation closed form (the 10-substep Euler recurrence is
    # linear: creep_10 = init_strain * (1 - (1-a)^10))
    ps = tf * 1.5 + 0.5
    dyc = py - center_y
    init_strain = ELONGATION * ps * np.abs(dyc) / (IMG_H / 2)
    em_eff = EM * (0.5 + tf * 0.8)
    visc_eff = VISC * (1.0 - tf * 0.5)
    a = (DT / 10.0) * (em_eff / visc_eff)
    creep = init_strain * (1.0 - (1.0 - a) ** 10)
    disp_y = creep * dyc * 0.1 * (ps * 3.0)
    disp_x = -0.7 * disp_y * 0.1
    bscale = np.where(np.asarray(boundary_flags, bool), 0.6, 1.0)
    disp_x = disp_x * bscale
    disp_y = disp_y * bscale

    Kx, Ky, KL, rowAx, rowAy, rowL = _stencil_tables(tf)

    tm_em = EM * (1.0 + tf * 0.3)
    max_iter = int(20 * (0.5 + tf * 0.5))
    n_diff = int(5 * (0.5 + tf * 0.5))
    c2 = DT / tm_em * 0.1          # scale of velocity in the d update
    # scaled-velocity formulation: S' = c2 * s, so
    #   S' += c2*DT*(-tm_em/10 * d + f),   d += S'
    fscale = c2 * DT               # folded into the conv weights
    e_grid_x = c2 * DT * (-tm_em / 10.0 - rowAx)   # [NY, NX]
    e_grid_y = c2 * DT * (-tm_em / 10.0 - rowAy)

    eff_d = DIFFUSION * (tf * 2.0 + 0.8)
    ed = eff_d * DT
    el_grid = -ed * rowL

    # --- V-frame tensors ([j partitions, i free]) ---
    PAD_F = R_F
    state0 = np.zeros((128, NY + 2 * PAD_F))
    state0[0:64, PAD_F:PAD_F + NY] = disp_x.reshape(NY, NX).T
    state0[64:128, PAD_F:PAD_F + NY] = disp_y.reshape(NY, NX).T

    ecoef = np.zeros((128, NY))
    ecoef[0:64] = e_grid_x.T
    ecoef[64:128] = e_grid_y.T

    # conv weights: block-diag(Bx_di, By_di), scaled by fscale, laid out as
    # [128, 9*128] with block b for di = b - R_F
    w9 = np.zeros((128, 9 * 128))
    # B[j, j'] = K(di, j - j'); K grid index is (j - j') + R
    for b, di in enumerate(range(-R_F, R_F + 1)):
        Bx = _banded(lambda dj: Kx[di + R_F, dj + R_F] if abs(dj) <= R_F else 0.0,
                     64, R_F)
        By = _banded(lambda dj: Ky[di + R_F, dj + R_F] if abs(dj) <= R_F else 0.0,
                     64, R_F)
        w9[0:64, b * 128:b * 128 + 64] = fscale * Bx
        w9[64:128, b * 128 + 64:b * 128 + 128] = fscale * By

    PAD_L = R_L
    m_init = (0.5 + tf * 0.2) - 0.1 + 0.2 * np.asarray(moisture_u, np.float64)
    m0 = np.zeros((64, NY + 2 * PAD_L))
    m0[:, PAD_L:PAD_L + NY] = m_init.reshape(NY, NX).T

    elcoef = np.zeros((64, NY))
    elcoef[:] = el_grid.T

    wl = np.zeros((64, 5 * 64))
    for b, di in enumerate(range(-R_L, R_L + 1)):
        BL = _banded(lambda dj: KL[di + R_L, dj + R_L] if abs(dj) <= R_L else 0.0,
                     64, R_L)
        wl[:, b * 64:b * 64 + 64] = ed * BL

    base = 0.5 + tf * 0.2
    beta = 0.2 * (1.0 + tf * 0.3)
    pcoup = 0.5 * (1.0 + tf * 0.2)
    tscale = 1.5 + tf * 2.0

    # pack the f32 constants + initial state into one blob so a single DMA
    # (= single semaphore) covers them all
    selsum = np.zeros((128, 64))
    selsum[0:64] = np.eye(64)
    selsum[64:128] = np.eye(64)
    parts = [("state", state0), ("ecoef", ecoef), ("m0", m0), ("el", elcoef),
             ("ident", np.eye(128)), ("sel", selsum)]
    off = 0
    offsets = {}
    for name, arr in parts:
        offsets[name] = off
        off += arr.shape[1]
    blob = np.zeros((128, off))
    for name, arr in parts:
        blob[0:arr.shape[0], offsets[name]:offsets[name] + arr.shape[1]] = arr

    # fp16 blob: conv weights (PE runs 2-byte matmuls ~4x faster than f32;
    # fp16 conv inputs keep the final rel err ~2e-4) + initial fp16 states
    # (precomputed here so no on-device init copies are needed)
    wf16 = np.zeros((128, 9 * 128 + 5 * 64 + (NY + 2 * R_F) + (NY + 2 * R_L)))
    wf16[:, 0:9 * 128] = w9
    wf16[0:64, 9 * 128:9 * 128 + 5 * 64] = wl
    s16o = 9 * 128 + 5 * 64
    m16o = s16o + NY + 2 * R_F
    wf16[:, s16o:s16o + NY + 2 * R_F] = state0.astype(np.float16)
    wf16[0:64, m16o:m16o + NY + 2 * R_L] = m0.astype(np.float16)

    arrays = {"blob": np.ascontiguousarray(blob, np.float32),
              "wf16": np.ascontiguousarray(wf16, np.float16)}
    scalars = dict(max_iter=max_iter, n_diff=n_diff, base=float(base),
                   bt=float(beta * tscale), pcoup=float(pcoup),
                   offsets=offsets, blob_cols=off)
    return arrays, scalars


# ---------------------------------------------------------------------------
# Bass program
# ---------------------------------------------------------------------------

def _build_nc(scalars):
    import concourse.bass as bass
    import concourse.mybir as mybir
    import concourse.tile as tile

    F32 = mybir.dt.float32
    F16 = mybir.dt.float16
    ALU = mybir.AluOpType
    max_iter = scalars["max_iter"]
    n_diff = scalars["n_diff"]
    off = scalars["offsets"]
    COLS = scalars["blob_cols"]
    NPF = NY + 2 * R_F
    NPL = NY + 2 * R_L
    S16O = 9 * 128 + 5 * 64
    M16O = S16O + NPF
    WCOLS = M16O + NPL

    nc = bass.Bass()
    p_blob = nc.declare_dram_parameter("blob", [128, COLS], F32, isOutput=False)
    p_w16 = nc.declare_dram_parameter("wf16", [128, WCOLS], F16, isOutput=False)
    p_out = nc.declare_dram_parameter("out", [2, NY, NX], F32, isOutput=True)

    SF = off["state"] + R_F   # interior start of the padded displacement state
    SL = off["m0"] + R_L      # interior start of the padded moisture state

    with (
        tile.TileContext(nc) as tc,
        tc.tile_pool(name="const", bufs=1) as cpool,
        tc.tile_pool(name="state", bufs=1) as spool,
        tc.tile_pool(name="work", bufs=2) as wpool,
        # bufs=2 so the next iteration's convs start while this iteration's
        # velocity update still reads the previous PSUM slot (multi-wait
        # instructions are legalized by _split_multiwait_ctrl)
        tc.tile_pool(name="psum", bufs=2, space="PSUM") as ppool,
        tc.tile_pool(name="epsum", bufs=1, space="PSUM") as epool,
    ):
        # one DMA per dtype; the mutable states live inside the f32 blob tile
        # (range-tracked deps)
        t_b = cpool.tile([128, COLS], F32, tag="blob")
        nc.sync.dma_start(t_b[:], p_blob[:])
        t_w = cpool.tile([128, WCOLS], F16, tag="wf16")
        nc.sync.dma_start(t_w[:], p_w16[:])

        def W9(b):
            return t_w[:, b * 128:(b + 1) * 128]

        def WL(b):
            return t_w[0:64, 9 * 128 + b * 64: 9 * 128 + (b + 1) * 64]

        st_int = t_b[:, SF:SF + NY]              # displacement state (V-frame)
        m_int = t_b[0:64, SL:SL + NY]            # moisture state (V-frame)
        e_int = t_b[:, off["ecoef"]:off["ecoef"] + NY]
        el_int = t_b[0:64, off["el"]:off["el"] + NY]

        t_vel = spool.tile([128, NY], F32, tag="vel")
        nc.vector.memset(t_vel[:], 0.0)

        # di order: 0 first so the first matmul initializes the full PSUM tile
        dis = [0] + [d for d in range(-R_F, R_F + 1) if d != 0]
        dls = [0] + [d for d in range(-R_L, R_L + 1) if d != 0]

        # Persistent fp16 shadows of the states, DMA'd in with their initial
        # values (pads zero); interiors rewritten in place each iteration.
        def S16(a, b):
            return t_w[:, S16O + a:S16O + b]

        def M16(a, b):
            return t_w[0:64, M16O + a:M16O + b]

        # Iteration dataflow (critical path = conv matmuls -> ONE fused DVE
        # op writing the next fp16 state -> next conv; the f32 state update
        # and velocity update overlap the next conv):
        #   t1    = ecoef * state         [during convs]
        #   vel  += t1                    [during convs]
        #   stp   = state + vel           [during convs]
        #   s16'  = f16(stp + psum)       [critical]
        #   state = stp + psum            [off critical path]
        #   vel  += psum                  [off critical path]
        for it in range(max_iter):
            ps = ppool.tile([128, NY], F32, tag="fpsum")
            for k, di in enumerate(dis):
                nc.tensor.matmul(
                    ps[:], W9(di + R_F), S16(R_F + di, R_F + di + NY),
                    start=(k == 0), stop=(k == len(dis) - 1),
                )
            t1 = wpool.tile([128, NY], F32, tag="t1")
            nc.vector.tensor_mul(t1[:], e_int, st_int)
            nc.vector.tensor_add(t_vel[:], t_vel[:], t1[:])
            stp = wpool.tile([128, NY], F32, tag="stp")
            nc.vector.tensor_add(stp[:], st_int, t_vel[:])
            if it < max_iter - 1:
                nc.vector.tensor_add(S16(R_F, R_F + NY), stp[:], ps[:])
            nc.vector.tensor_add(st_int, stp[:], ps[:])
            nc.vector.tensor_add(t_vel[:], t_vel[:], ps[:])

        for it in range(n_diff):
            psL = ppool.tile([64, NY], F32, tag="lpsum")
            for k, di in enumerate(dls):
                nc.tensor.matmul(
                    psL[:], WL(di + R_L), M16(R_L + di, R_L + di + NY),
                    start=(k == 0), stop=(k == len(dls) - 1),
                )
            tL = wpool.tile([64, NY], F32, tag="tL")
            nc.vector.tensor_mul(tL[:], el_int, m_int)
            mp = wpool.tile([64, NY], F32, tag="mp")
            nc.vector.tensor_add(mp[:], m_int, tL[:])
            if it < n_diff - 1:
                nc.vector.tensor_add(M16(R_L, R_L + NY), mp[:], psL[:])
            nc.vector.tensor_add(m_int, mp[:], psL[:])

        # ---- epilogue: moisture coupling, PE transposes, store ----
        ident = t_b[:, off["ident"]:off["ident"] + 128]
        negd = wpool.tile([128, NY], F32, tag="negd")
        nc.vector.tensor_scalar_mul(negd[:], st_int, -1.0)
        absd = wpool.tile([128, NY], F32, tag="absd")
        nc.vector.tensor_max(absd[:], st_int, negd[:])
        ps_s = epool.tile([64, NY], F32, tag="eps_s")
        nc.tensor.matmul(ps_s[:], t_b[:, off["sel"]:off["sel"] + 64], absd[:])
        ps_t = epool.tile([NY, 128], F32, tag="eps_t")
        nc.tensor.transpose(ps_t[:], st_int, ident)

        sf = wpool.tile([64, NY], F32, tag="sf")
        nc.vector.tensor_scalar(sf[:], ps_s[:], scalars["pcoup"], 1.0, ALU.mult, ALU.add)
        ms = wpool.tile([64, NY], F32, tag="ms")
        nc.vector.tensor_scalar(ms[:], m_int, scalars["base"],
                                scalars["bt"], ALU.subtract, ALU.mult)
        mdx = wpool.tile([64, NY], F32, tag="mdx")
        nc.vector.tensor_mul(mdx[:], ms[:], sf[:])
        ps_m = epool.tile([NY, 64], F32, tag="eps_m")
        nc.tensor.transpose(ps_m[:], mdx[:], ident[0:64, 0:64])

        s_norm = wpool.tile([NY, 128], F32, tag="s_norm")
        nc.vector.tensor_copy(s_norm[:], ps_t[:])
        mdx_n = wpool.tile([NY, 64], F32, tag="mdx_n")
        nc.vector.tensor_copy(mdx_n[:], ps_m[:])
        mdx2 = wpool.tile([NY, 64], F32, tag="mdx2")
        nc.vector.tensor_scalar_mul(mdx2[:], mdx_n[:], 2.0)
        fin = wpool.tile([NY, 128], F32, tag="fin")
        nc.vector.tensor_add(fin[:, 0:64], s_norm[:, 0:64], mdx_n[:])
        nc.vector.tensor_add(fin[:, 64:128], s_norm[:, 64:128], mdx2[:])

        nc.sync.dma_start(p_out.rearrange("c i j -> i c j"), fin[:])

    _split_multiwait_ctrl(nc)
    return nc


def _split_multiwait_ctrl(nc, maxw=1):
    """This walrus build encodes only one sync wait per instruction.  For any
    instruction carrying more, insert same-engine InstDrain wait-carriers
    immediately before it (the engine executes them in order, so the union of
    waits is preserved; a drain merely flushes that engine's pipeline)."""
    import concourse.mybir as mybir

    for fn in nc.m.functions:
        for blk in fn.blocks:
            insts = getattr(blk, "instructions", None)
            if not insts:
                continue
            i = 0
            while i < len(insts):
                inst = insts[i]
                si = inst.sync_info
                if (si is not None and si.on_wait and len(si.on_wait) > maxw
                        and inst.engine is not None):
                    waits = list(si.on_wait)
                    for j, w in enumerate(waits[:-maxw]):
                        d = mybir.InstDrain(name=f"{inst.name}-w{j}", ins=[], outs=[])
                        d.engine = inst.engine
                        d.sync_info = mybir.SyncInfo(on_wait=[w], on_update=[])
                        try:
                            nc.register_instruction(d, overwrite=True)
                        except Exception:
                            pass
                        insts.insert(i, d)
                        i += 1
                    inst.sync_info = mybir.SyncInfo(
                        on_wait=waits[-maxw:], on_update=list(si.on_update))
                i += 1


# ---------------------------------------------------------------------------
# dense numpy fallback (only if points are not the expected uniform grid)
# ---------------------------------------------------------------------------

def _numpy_dense(points, boundary_flags, moisture_u, t, total_steps):
    tf = float(t) / float(total_steps)
    pts = np.asarray(points, np.float32)
    px, py = pts[:, 0].astype(np.float64), pts[:, 1].astype(np.float64)
    center_y = py.mean()
    ps = tf * 1.5 + 0.5
    dyc = py - center_y
    init_strain = ELONGATION * ps * np.abs(dyc) / (IMG_H / 2)
    a = (DT / 10.0) * (EM * (0.5 + tf * 0.8) / (VISC * (1.0 - tf * 0.5)))
    creep = init_strain * (1.0 - (1.0 - a) ** 10)
    disp_y = creep * dyc * 0.1 * (ps * 3.0)
    disp_x = -0.7 * disp_y * 0.1
    bscale = np.where(np.asarray(boundary_flags, bool), 0.6, 1.0)
    disp_x = (disp_x * bscale).astype(np.float32)
    disp_y = (disp_y * bscale).astype(np.float32)

    dxm = px[None, :] - px[:, None]
    dym = py[None, :] - py[:, None]
    dist = np.sqrt(dxm * dxm + dym * dym)
    eye = np.eye(len(px), dtype=bool)
    pc = COUPLING * (1.0 + tf * 0.5)
    C = np.where((dist < 50.0) & ~eye, pc * np.exp(-dist / 20.0), 0.0)
    safe = np.where(dist > 0, dist, 1.0)
    W = C * EM * (1.0 + tf)
    Ax = (W * dxm / (safe * safe)).astype(np.float32)
    Ay = (W * dym / (safe * safe)).astype(np.float32)
    rowAx = Ax.sum(axis=1)
    rowAy = Ay.sum(axis=1)

    tm_em = np.float32(EM * (1.0 + tf * 0.3))
    sx = np.zeros_like(disp_x)
    sy = np.zeros_like(disp_y)
    dx_, dy_ = disp_x.copy(), disp_y.copy()
    for _ in range(int(20 * (0.5 + tf * 0.5))):
        fx = Ax @ dx_ - rowAx * dx_
        fy = Ay @ dy_ - rowAy * dy_
        sx = (sx + DT * (-tm_em * dx_ / 10.0 + fx)).astype(np.float32)
        sy = (sy + DT * (-tm_em * dy_ / 10.0 + fy)).astype(np.float32)
        dx_ = (dx_ + DT * sx / tm_em * 0.1).astype(np.float32)
        dy_ = (dy_ + DT * sy / tm_em * 0.1).astype(np.float32)

    Loff = np.where((dist < 30.0) & ~eye, 1.0 / (dist + 1e-6), 0.0).astype(np.float32)
    L = Loff - np.diag(Loff.sum(axis=1))
    base = 0.5 + tf * 0.2
    m = (base - 0.1 + 0.2 * np.asarray(moisture_u, np.float64)).astype(np.float32)
    eff_d = np.float32(DIFFUSION * (tf * 2.0 + 0.8))
    for _ in range(int(5 * (0.5 + tf * 0.5))):
        m = (m + eff_d * DT * (L @ m)).astype(np.float32)
    beta = 0.2 * (1.0 + tf * 0.3)
    pcoup = 0.5 * (1.0 + tf * 0.2)
    tscale = 1.5 + tf * 2.0
    strain = beta * (m - base)
    sf = 1.0 + pcoup * (np.abs(dx_) + np.abs(dy_))
    mdx = strain * sf * 1.0 * tscale
    mdy = strain * sf * 2.0 * tscale
    return np.stack([dx_ + mdx, dy_ + mdy]).astype(np.float32)


# ---------------------------------------------------------------------------
# entry point
# ---------------------------------------------------------------------------

def kernel(points, boundary_flags, moisture_u, t, total_steps):
    points = np.asarray(points, np.float32)
    boundary_flags = np.asarray(boundary_flags, bool)
    moisture_u = np.asarray(moisture_u, np.float32)
    t = int(np.asarray(t)); total_steps = int(np.asarray(total_steps))

    on_grid = (points.shape == (N, 2) and
               float(np.abs(points - _grid_points()).max()) < 1e-3)
    if not on_grid:
        return _numpy_dense(points, boundary_flags, moisture_u, t, total_steps)

    arrays, scalars = _host_constants(points, boundary_flags, moisture_u, t, total_steps)
    nc = _build_nc(scalars)

    from concourse.bass_utils import run_bass_kernel_spmd
    in_maps = [dict(arrays) for _ in range(N_CORES)]
    kwargs = dict(TRACE_KWARGS)  # test harness can enable tracing
    res = run_bass_kernel_spmd(nc, in_maps, list(range(N_CORES)), **kwargs)
    globals()["LAST_RESULT"] = res
    out = np.asarray(res.results[0]["out"], np.float32)
    return out.reshape(2, N)


TRACE_KWARGS: dict = {}
LAST_RESULT = None


# revision 30
# speedup vs baseline: 1.1052x; 1.1052x over previous
"""Trainium2 Bass kernel for nn_CreepDiffusionTrainer.

The reference builds dense NxN (6144x6144) pairwise coupling / laplacian
matrices from points that lie on a uniform 96x64 grid.  On that grid every
pairwise quantity depends only on the integer offset (di, dj), so the dense
matvecs are exactly 2D convolutions with small stencils:

  - coupling (dist < 50):  68 taps inside a 9x9 window
  - laplacian (dist < 30):  24 taps inside a 5x5 window

(nearest excluded offsets are >= 0.28 px away from the thresholds, far beyond
f32 rounding of the coordinates, so the masks are identical).

The kernel keeps the state in a transposed "V-frame" layout: SBUF tiles of
[64 partitions = x-column j, 96 free = y-row i], with the x- and y-systems
stacked into one [128, 96] tile.  Each conv is 9 PSUM-accumulated matmuls
out[j',i] += sum_j B_di[j, j'] * state[j, i+di] with constant banded weight
matrices B_di (the dj-part of the stencil) and free-dim-shifted reads of a
zero-padded state tile (the di-part).  Row sums, the creep/elongation init
and all scalar schedules are folded into small constant tensors on the host.

The same program runs replicated on all 8 NeuronCores (the whole problem is
~30us of latency-bound work; sharding 96 rows across cores would only add
per-iteration collective latency), and core 0's output is returned.

If the points ever do NOT form the expected uniform grid, a dense numpy
fallback that mirrors the reference math exactly is used instead.
"""

import numpy as np

IMG_H, IMG_W = 960, 640
NY, NX = 96, 64
N = NY * NX
DT = 1.0
ELONGATION = 0.15
COUPLING = 0.3
DIFFUSION = 0.1
EM = 1.0
VISC = 10.0
HX = (IMG_W - 1) / (NX - 1)
HY = (IMG_H - 1) / (NY - 1)
R_F = 4   # force stencil radius (dist < 50)
R_L = 2   # laplacian stencil radius (dist < 30)
N_CORES = 8


# ---------------------------------------------------------------------------
# host-side constant construction (all f64, cast to f32 at the end)
# ---------------------------------------------------------------------------

def _grid_points():
    xs = np.linspace(0.0, IMG_W - 1, NX)
    ys = np.linspace(0.0, IMG_H - 1, NY)
    Y, X = np.meshgrid(ys, xs, indexing="ij")
    return np.stack([X.ravel(), Y.ravel()], axis=-1)


def _correlate2d_zero(U, K, R):
    out = np.zeros_like(U)
    ny, nx = U.shape
    for di in range(-R, R + 1):
        for dj in range(-R, R + 1):
            k = K[di + R, dj + R]
            if k == 0.0:
                continue
            i0, i1 = max(0, -di), ny - max(0, di)
            j0, j1 = max(0, -dj), nx - max(0, dj)
            out[i0:i1, j0:j1] += k * U[i0 + di:i1 + di, j0 + dj:j1 + dj]
    return out


def _stencil_tables(tf):
    pc = COUPLING * (1.0 + tf * 0.5)
    wc = pc * EM * (1.0 + tf)
    Kx = np.zeros((2 * R_F + 1, 2 * R_F + 1))
    Ky = np.zeros_like(Kx)
    KL = np.zeros((2 * R_L + 1, 2 * R_L + 1))
    for di in range(-R_F, R_F + 1):
        for dj in range(-R_F, R_F + 1):
            d = float(np.hypot(di * HY, dj * HX))
            if 0.0 < d < 50.0:
                w = wc * np.exp(-d / 20.0)
                Kx[di + R_F, dj + R_F] = w * (dj * HX) / (d * d)
                Ky[di + R_F, dj + R_F] = w * (di * HY) / (d * d)
            if 0.0 < d < 30.0 and abs(di) <= R_L and abs(dj) <= R_L:
                KL[di + R_L, dj + R_L] = 1.0 / (d + 1e-6)
    ones = np.ones((NY, NX))
    rowAx = _correlate2d_zero(ones, Kx, R_F)
    rowAy = _correlate2d_zero(ones, Ky, R_F)
    rowL = _correlate2d_zero(ones, KL, R_L)
    return Kx, Ky, KL, rowAx, rowAy, rowL


def _banded(K1d_getter, n, band_r):
    """B[j, j'] = K(j - j') as an n x n matrix."""
    B = np.zeros((n, n))
    for j in range(n):
        for jp in range(max(0, j - band_r), min(n, j + band_r + 1)):
            B[j, jp] = K1d_getter(j - jp)
    return B


def _host_constants(points, boundary_flags, moisture_u, t, total_steps):
    tf = float(t) / float(total_steps)
    py = points[:, 1].astype(np.float64)
    center_y = py.mean()

    # creep / elongation closed form (the 10-substep Euler recurrence is
    # linear: creep_10 = init_strain * (1 - (1-a)^10))
    ps = tf * 1.5 + 0.5
    dyc = py - center_y
    init_strain = ELONGATION * ps * np.abs(dyc) / (IMG_H / 2)
    em_eff = EM * (0.5 + tf * 0.8)
    visc_eff = VISC * (1.0 - tf * 0.5)
    a = (DT / 10.0) * (em_eff / visc_eff)
    creep = init_strain * (1.0 - (1.0 - a) ** 10)
    disp_y = creep * dyc * 0.1 * (ps * 3.0)
    disp_x = -0.7 * disp_y * 0.1
    bscale = np.where(np.asarray(boundary_flags, bool), 0.6, 1.0)
    disp_x = disp_x * bscale
    disp_y = disp_y * bscale

    Kx, Ky, KL, rowAx, rowAy, rowL = _stencil_tables(tf)

    tm_em = EM * (1.0 + tf * 0.3)
    max_iter = int(20 * (0.5 + tf * 0.5))
    n_diff = int(5 * (0.5 + tf * 0.5))
    c2 = DT / tm_em * 0.1          # scale of velocity in the d update
    # scaled-velocity formulation: S' = c2 * s, so
    #   S' += c2*DT*(-tm_em/10 * d + f),   d += S'
    fscale = c2 * DT               # folded into the conv weights
    e_grid_x = c2 * DT * (-tm_em / 10.0 - rowAx)   # [NY, NX]
    e_grid_y = c2 * DT * (-tm_em / 10.0 - rowAy)

    eff_d = DIFFUSION * (tf * 2.0 + 0.8)
    ed = eff_d * DT
    el_grid = -ed * rowL

    # --- V-frame tensors ([j partitions, i free]) ---
    PAD_F = R_F
    state0 = np.zeros((128, NY + 2 * PAD_F))
    state0[0:64, PAD_F:PAD_F + NY] = disp_x.reshape(NY, NX).T
    state0[64:128, PAD_F:PAD_F + NY] = disp_y.reshape(NY, NX).T

    # two-term recurrence: state_{k+1} = (2+E) . state_k - state_{k-1} + ps_k
    # (vel_k == state_k - state_{k-1}); iteration 0 uses (1+E) with no prev
    e1coef = np.zeros((128, NY))
    e1coef[0:64] = 1.0 + e_grid_x.T
    e1coef[64:128] = 1.0 + e_grid_y.T
    e2coef = e1coef + 1.0

    # conv weights: block-diag(Bx_di, By_di), scaled by fscale, laid out as
    # [128, 9*128] with block b for di = b - R_F
    w9 = np.zeros((128, 9 * 128))
    # B[j, j'] = K(di, j - j'); K grid index is (j - j') + R
    for b, di in enumerate(range(-R_F, R_F + 1)):
        Bx = _banded(lambda dj: Kx[di + R_F, dj + R_F] if abs(dj) <= R_F else 0.0,
                     64, R_F)
        By = _banded(lambda dj: Ky[di + R_F, dj + R_F] if abs(dj) <= R_F else 0.0,
                     64, R_F)
        w9[0:64, b * 128:b * 128 + 64] = fscale * Bx
        w9[64:128, b * 128 + 64:b * 128 + 128] = fscale * By

    PAD_L = R_L
    m_init = (0.5 + tf * 0.2) - 0.1 + 0.2 * np.asarray(moisture_u, np.float64)
    m0 = np.zeros((64, NY + 2 * PAD_L))
    m0[:, PAD_L:PAD_L + NY] = m_init.reshape(NY, NX).T

    elcoef = np.zeros((64, NY))
    elcoef[:] = 1.0 + el_grid.T     # m' = (1+EL) . m + psL

    wl = np.zeros((64, 5 * 64))
    for b, di in enumerate(range(-R_L, R_L + 1)):
        BL = _banded(lambda dj: KL[di + R_L, dj + R_L] if abs(dj) <= R_L else 0.0,
                     64, R_L)
        wl[:, b * 64:b * 64 + 64] = ed * BL

    base = 0.5 + tf * 0.2
    beta = 0.2 * (1.0 + tf * 0.3)
    pcoup = 0.5 * (1.0 + tf * 0.2)
    tscale = 1.5 + tf * 2.0

    # pack the f32 constants + initial state into one blob so a single DMA
    # (= single semaphore) covers them all
    selsum = np.zeros((128, 64))
    selsum[0:64] = np.eye(64)
    selsum[64:128] = np.eye(64)
    parts = [("state", state0), ("e1", e1coef), ("e2", e2coef),
             ("m0", m0), ("el", elcoef),
             ("ident", np.eye(128)), ("sel", selsum)]
    off = 0
    offsets = {}
    for name, arr in parts:
        offsets[name] = off
        off += arr.shape[1]
    blob = np.zeros((128, off))
    for name, arr in parts:
        blob[0:arr.shape[0], offsets[name]:offsets[name] + arr.shape[1]] = arr

    # fp16 blob: conv weights (PE runs 2-byte matmuls ~4x faster than f32;
    # fp16 conv inputs keep the final rel err ~2e-4) + initial fp16 states
    # (precomputed here so no on-device init copies are needed)
    wf16 = np.zeros((128, 9 * 128 + 5 * 64 + (NY + 2 * R_F) + (NY + 2 * R_L)))
    wf16[:, 0:9 * 128] = w9
    wf16[0:64, 9 * 128:9 * 128 + 5 * 64] = wl
    s16o = 9 * 128 + 5 * 64
    m16o = s16o + NY + 2 * R_F
    wf16[:, s16o:s16o + NY + 2 * R_F] = state0.astype(np.float16)
    wf16[0:64, m16o:m16o + NY + 2 * R_L] = m0.astype(np.float16)

    arrays = {"blob": np.ascontiguousarray(blob, np.float32),
              "wf16": np.ascontiguousarray(wf16, np.float16)}
    scalars = dict(max_iter=max_iter, n_diff=n_diff, base=float(base),
                   bt=float(beta * tscale), pcoup=float(pcoup),
                   offsets=offsets, blob_cols=off)
    return arrays, scalars


# ---------------------------------------------------------------------------
# Bass program
# ---------------------------------------------------------------------------

def _build_nc(scalars):
    import concourse.bass as bass
    import concourse.mybir as mybir
    import concourse.tile as tile

    F32 = mybir.dt.float32
    F16 = mybir.dt.float16
    ALU = mybir.AluOpType
    max_iter = scalars["max_iter"]
    n_diff = scalars["n_diff"]
    off = scalars["offsets"]
    COLS = scalars["blob_cols"]
    NPF = NY + 2 * R_F
    NPL = NY + 2 * R_L
    S16O = 9 * 128 + 5 * 64
    M16O = S16O + NPF
    WCOLS = M16O + NPL

    nc = bass.Bass()
    p_blob = nc.declare_dram_parameter("blob", [128, COLS], F32, isOutput=False)
    p_w16 = nc.declare_dram_parameter("wf16", [128, WCOLS], F16, isOutput=False)
    p_out = nc.declare_dram_parameter("out", [2, NY, NX], F32, isOutput=True)

    SF = off["state"] + R_F   # interior start of the padded displacement state
    SL = off["m0"] + R_L      # interior start of the padded moisture state

    with (
        tile.TileContext(nc) as tc,
        tc.tile_pool(name="const", bufs=1) as cpool,
        tc.tile_pool(name="state", bufs=1) as spool,
        tc.tile_pool(name="work", bufs=2) as wpool,
        # bufs=2 so the next iteration's convs start while this iteration's
        # velocity update still reads the previous PSUM slot (multi-wait
        # instructions are legalized by _split_multiwait_ctrl)
        tc.tile_pool(name="psum", bufs=2, space="PSUM") as ppool,
        tc.tile_pool(name="epsum", bufs=1, space="PSUM") as epool,
    ):
        # one DMA per dtype; the mutable states live inside the f32 blob tile
        # (range-tracked deps)
        t_b = cpool.tile([128, COLS], F32, tag="blob")
        nc.sync.dma_start(t_b[:], p_blob[:])
        t_w = cpool.tile([128, WCOLS], F16, tag="wf16")
        nc.sync.dma_start(t_w[:], p_w16[:])

        def W9(b):
            return t_w[:, b * 128:(b + 1) * 128]

        def WL(b):
            return t_w[0:64, 9 * 128 + b * 64: 9 * 128 + (b + 1) * 64]

        st_int = t_b[:, SF:SF + NY]              # displacement state (V-frame)
        m_int = t_b[0:64, SL:SL + NY]            # moisture state (V-frame)
        e1_int = t_b[:, off["e1"]:off["e1"] + NY]
        e2_int = t_b[:, off["e2"]:off["e2"] + NY]
        el_int = t_b[0:64, off["el"]:off["el"] + NY]

        t_stB = spool.tile([128, NY], F32, tag="stB")

        # di order: 0 first so the first matmul initializes the full PSUM tile
        dis = [0] + [d for d in range(-R_F, R_F + 1) if d != 0]
        dls = [0] + [d for d in range(-R_L, R_L + 1) if d != 0]

        # Persistent fp16 shadows of the states, DMA'd in with their initial
        # values (pads zero); interiors rewritten in place each iteration.
        def S16(a, b):
            return t_w[:, S16O + a:S16O + b]

        def M16(a, b):
            return t_w[0:64, M16O + a:M16O + b]

        # Iteration dataflow (two-term recurrence, no velocity tensor):
        #   stp   = (2+E).state_k - state_{k-1}   [2 ops during convs;
        #                                          iter 0: (1+E).state_0]
        #   s16'  = f16(stp + psum)               [critical]
        #   state_{k+1} = stp + psum              [ping-pong, off crit. path]
        cur, prev = st_int, t_stB[:]
        for it in range(max_iter):
            ps = ppool.tile([128, NY], F32, tag="fpsum")
            for k, di in enumerate(dis):
                nc.tensor.matmul(
                    ps[:], W9(di + R_F), S16(R_F + di, R_F + di + NY),
                    start=(k == 0), stop=(k == len(dis) - 1),
                )
            stp = wpool.tile([128, NY], F32, tag="stp")
            if it == 0:
                nc.vector.tensor_mul(stp[:], e1_int, cur)
            else:
                t2 = wpool.tile([128, NY], F32, tag="t2")
                nc.vector.tensor_mul(t2[:], e2_int, cur)
                nc.vector.tensor_sub(stp[:], t2[:], prev)
            if it < max_iter - 1:
                nc.vector.tensor_add(S16(R_F, R_F + NY), stp[:], ps[:])
            nc.vector.tensor_add(prev if it > 0 else t_stB[:], stp[:], ps[:])
            if it == 0:
                cur, prev = t_stB[:], st_int
            else:
                cur, prev = prev, cur

        for it in range(n_diff):
            psL = ppool.tile([64, NY], F32, tag="lpsum")
            for k, di in enumerate(dls):
                nc.tensor.matmul(
                    psL[:], WL(di + R_L), M16(R_L + di, R_L + di + NY),
                    start=(k == 0), stop=(k == len(dls) - 1),
                )
            mp = wpool.tile([64, NY], F32, tag="mp")
            nc.vector.tensor_mul(mp[:], el_int, m_int)
            if it < n_diff - 1:
                nc.vector.tensor_add(M16(R_L, R_L + NY), mp[:], psL[:])
            nc.vector.tensor_add(m_int, mp[:], psL[:])

        # ---- epilogue: moisture coupling, PE transposes, store ----
        ident = t_b[:, off["ident"]:off["ident"] + 128]
        negd = wpool.tile([128, NY], F32, tag="negd")
        nc.vector.tensor_scalar_mul(negd[:], cur, -1.0)
        absd = wpool.tile([128, NY], F32, tag="absd")
        nc.vector.tensor_max(absd[:], cur, negd[:])
        ps_s = epool.tile([64, NY], F32, tag="eps_s")
        nc.tensor.matmul(ps_s[:], t_b[:, off["sel"]:off["sel"] + 64], absd[:])
        ps_t = epool.tile([NY, 128], F32, tag="eps_t")
        nc.tensor.transpose(ps_t[:], cur, ident)

        sf = wpool.tile([64, NY], F32, tag="sf")
        nc.vector.tensor_scalar(sf[:], ps_s[:], scalars["pcoup"], 1.0, ALU.mult, ALU.add)
        ms = wpool.tile([64, NY], F32, tag="ms")
        nc.vector.tensor_scalar(ms[:], m_int, scalars["base"],
                                scalars["bt"], ALU.subtract, ALU.mult)
        mdx = wpool.tile([64, NY], F32, tag="mdx")
        nc.vector.tensor_mul(mdx[:], ms[:], sf[:])
        ps_m = epool.tile([NY, 64], F32, tag="eps_m")
        nc.tensor.transpose(ps_m[:], mdx[:], ident[0:64, 0:64])

        s_norm = wpool.tile([NY, 128], F32, tag="s_norm")
        nc.vector.tensor_copy(s_norm[:], ps_t[:])
        mdx_n = wpool.tile([NY, 64], F32, tag="mdx_n")
        nc.vector.tensor_copy(mdx_n[:], ps_m[:])
        mdx2 = wpool.tile([NY, 64], F32, tag="mdx2")
        nc.vector.tensor_scalar_mul(mdx2[:], mdx_n[:], 2.0)
        fin = wpool.tile([NY, 128], F32, tag="fin")
        nc.vector.tensor_add(fin[:, 0:64], s_norm[:, 0:64], mdx_n[:])
        nc.vector.tensor_add(fin[:, 64:128], s_norm[:, 64:128], mdx2[:])

        nc.sync.dma_start(p_out.rearrange("c i j -> i c j"), fin[:])

    _split_multiwait_ctrl(nc)
    return nc


def _split_multiwait_ctrl(nc, maxw=1):
    """This walrus build encodes only one sync wait per instruction.  For any
    instruction carrying more, insert same-engine InstDrain wait-carriers
    immediately before it (the engine executes them in order, so the union of
    waits is preserved; a drain merely flushes that engine's pipeline)."""
    import concourse.mybir as mybir

    for fn in nc.m.functions:
        for blk in fn.blocks:
            insts = getattr(blk, "instructions", None)
            if not insts:
                continue
            i = 0
            while i < len(insts):
                inst = insts[i]
                si = inst.sync_info
                if (si is not None and si.on_wait and len(si.on_wait) > maxw
                        and inst.engine is not None):
                    waits = list(si.on_wait)
                    for j, w in enumerate(waits[:-maxw]):
                        d = mybir.InstDrain(name=f"{inst.name}-w{j}", ins=[], outs=[])
                        d.engine = inst.engine
                        d.sync_info = mybir.SyncInfo(on_wait=[w], on_update=[])
                        try:
                            nc.register_instruction(d, overwrite=True)
                        except Exception:
                            pass
                        insts.insert(i, d)
                        i += 1
                    inst.sync_info = mybir.SyncInfo(
                        on_wait=waits[-maxw:], on_update=list(si.on_update))
                i += 1


# ---------------------------------------------------------------------------
# dense numpy fallback (only if points are not the expected uniform grid)
# ---------------------------------------------------------------------------

def _numpy_dense(points, boundary_flags, moisture_u, t, total_steps):
    tf = float(t) / float(total_steps)
    pts = np.asarray(points, np.float32)
    px, py = pts[:, 0].astype(np.float64), pts[:, 1].astype(np.float64)
    center_y = py.mean()
    ps = tf * 1.5 + 0.5
    dyc = py - center_y
    init_strain = ELONGATION * ps * np.abs(dyc) / (IMG_H / 2)
    a = (DT / 10.0) * (EM * (0.5 + tf * 0.8) / (VISC * (1.0 - tf * 0.5)))
    creep = init_strain * (1.0 - (1.0 - a) ** 10)
    disp_y = creep * dyc * 0.1 * (ps * 3.0)
    disp_x = -0.7 * disp_y * 0.1
    bscale = np.where(np.asarray(boundary_flags, bool), 0.6, 1.0)
    disp_x = (disp_x * bscale).astype(np.float32)
    disp_y = (disp_y * bscale).astype(np.float32)

    dxm = px[None, :] - px[:, None]
    dym = py[None, :] - py[:, None]
    dist = np.sqrt(dxm * dxm + dym * dym)
    eye = np.eye(len(px), dtype=bool)
    pc = COUPLING * (1.0 + tf * 0.5)
    C = np.where((dist < 50.0) & ~eye, pc * np.exp(-dist / 20.0), 0.0)
    safe = np.where(dist > 0, dist, 1.0)
    W = C * EM * (1.0 + tf)
    Ax = (W * dxm / (safe * safe)).astype(np.float32)
    Ay = (W * dym / (safe * safe)).astype(np.float32)
    rowAx = Ax.sum(axis=1)
    rowAy = Ay.sum(axis=1)

    tm_em = np.float32(EM * (1.0 + tf * 0.3))
    sx = np.zeros_like(disp_x)
    sy = np.zeros_like(disp_y)
    dx_, dy_ = disp_x.copy(), disp_y.copy()
    for _ in range(int(20 * (0.5 + tf * 0.5))):
        fx = Ax @ dx_ - rowAx * dx_
        fy = Ay @ dy_ - rowAy * dy_
        sx = (sx + DT * (-tm_em * dx_ / 10.0 + fx)).astype(np.float32)
        sy = (sy + DT * (-tm_em * dy_ / 10.0 + fy)).astype(np.float32)
        dx_ = (dx_ + DT * sx / tm_em * 0.1).astype(np.float32)
        dy_ = (dy_ + DT * sy / tm_em * 0.1).astype(np.float32)

    Loff = np.where((dist < 30.0) & ~eye, 1.0 / (dist + 1e-6), 0.0).astype(np.float32)
    L = Loff - np.diag(Loff.sum(axis=1))
    base = 0.5 + tf * 0.2
    m = (base - 0.1 + 0.2 * np.asarray(moisture_u, np.float64)).astype(np.float32)
    eff_d = np.float32(DIFFUSION * (tf * 2.0 + 0.8))
    for _ in range(int(5 * (0.5 + tf * 0.5))):
        m = (m + eff_d * DT * (L @ m)).astype(np.float32)
    beta = 0.2 * (1.0 + tf * 0.3)
    pcoup = 0.5 * (1.0 + tf * 0.2)
    tscale = 1.5 + tf * 2.0
    strain = beta * (m - base)
    sf = 1.0 + pcoup * (np.abs(dx_) + np.abs(dy_))
    mdx = strain * sf * 1.0 * tscale
    mdy = strain * sf * 2.0 * tscale
    return np.stack([dx_ + mdx, dy_ + mdy]).astype(np.float32)


# ---------------------------------------------------------------------------
# entry point
# ---------------------------------------------------------------------------

def kernel(points, boundary_flags, moisture_u, t, total_steps):
    points = np.asarray(points, np.float32)
    boundary_flags = np.asarray(boundary_flags, bool)
    moisture_u = np.asarray(moisture_u, np.float32)
    t = int(np.asarray(t)); total_steps = int(np.asarray(total_steps))

    on_grid = (points.shape == (N, 2) and
               float(np.abs(points - _grid_points()).max()) < 1e-3)
    if not on_grid:
        return _numpy_dense(points, boundary_flags, moisture_u, t, total_steps)

    arrays, scalars = _host_constants(points, boundary_flags, moisture_u, t, total_steps)
    nc = _build_nc(scalars)

    from concourse.bass_utils import run_bass_kernel_spmd
    in_maps = [dict(arrays) for _ in range(N_CORES)]
    kwargs = dict(TRACE_KWARGS)  # test harness can enable tracing
    res = run_bass_kernel_spmd(nc, in_maps, list(range(N_CORES)), **kwargs)
    globals()["LAST_RESULT"] = res
    out = np.asarray(res.results[0]["out"], np.float32)
    return out.reshape(2, N)


TRACE_KWARGS: dict = {}
LAST_RESULT = None
